# revision 24
# baseline (speedup 1.0000x reference)
"""Trainium2 Bass kernel for Swin-3D shifted-window attention block.

Strategy: data-parallel over the 64 attention windows -> 8 windows per
NeuronCore, zero collectives (attention is per-window; everything else is
per-token). Host side does only input prep: the roll+window permutation of
tokens (sharding), the relative-position-bias gather, and exact algebraic
folds of layernorm affine / biases / attention scale into the GEMM weights.

Device-side pipeline per core (phase-major for ACT-table locality):
  P1: LN1 stats (token-major) -> z1 -> PE-transpose -> z1T (C-major)
  P3: per window: QK GEMM (C-major, 32-padded heads), Vext GEMM (token-major,
      [1|v|0] 32-strided head blocks), per head: S^T = kT.T@qT in PSUM,
      += bias+mask via identity-matmul (bf16), exp on ACT, AV via
      vext.T @ exp(S^T) giving outT with denominator channel, reciprocal +
      PE-broadcast + normalize, proj (token-major) + residual.
  P4: LN2 -> z2T.  P5: fc1 (C-major) -> gelu(+bias) -> fc2 (token-major,
      bias via ones-row of hT) -> + residual -> DMA out.

All PE matmuls run as float32r (full speed at N>=256); bias+mask ride bf16.
"""
import os
import sys

import numpy as np

for _p in ("/opt/trn_rl_repo", "/root/.axon_site/_ro/trn_rl_repo"):
    if os.path.isdir(_p) and _p not in sys.path:
        sys.path.insert(0, _p)

import ml_dtypes  # noqa: E402
import concourse.bass as bass  # noqa: E402
import concourse.bacc as bacc  # noqa: E402
import concourse.tile as tile  # noqa: E402
from concourse import mybir  # noqa: E402
from concourse import bass_utils  # noqa: E402

F32 = mybir.dt.float32
F32R = mybir.dt.float32r
BF16 = mybir.dt.bfloat16
AX = mybir.AxisListType
OP = mybir.AluOpType
AF = mybir.ActivationFunctionType

B, H, W, D, C = 1, 32, 32, 32, 180
WS, SHIFT, NH = 8, 4, 6
N = WS ** 3            # 512 tokens / window
HD = C // NH           # 30
NCORES = 8
NW = 64
WPC = NW // NCORES     # 8 windows / core
TPC = WPC * N          # 4096 tokens / core
NT = TPC // 128        # 32 token tiles / core
EPS = 1e-5
SCALE = HD ** -0.5
HID = 720              # mlp hidden


def _build_perm():
    idx = np.arange(H * W * D).reshape(H, W, D)
    idx = np.roll(idx, (-SHIFT, -SHIFT, -SHIFT), axis=(0, 1, 2))
    idx = idx.reshape(H // WS, WS, W // WS, WS, D // WS, WS)
    return idx.transpose(0, 2, 4, 1, 3, 5).reshape(NW * N)


def _host_prep(inputs):
    f32 = np.float32
    x = np.asarray(inputs['x'], f32).reshape(H * W * D, C)
    rpi = np.asarray(inputs['rpi_sa'])
    mask = np.asarray(inputs['attn_mask'], f32)
    g1 = np.asarray(inputs['norm1_w'], f32); b1 = np.asarray(inputs['norm1_b'], f32)
    qkv_w = np.asarray(inputs['qkv_w'], f32); qkv_b = np.asarray(inputs['qkv_b'], f32)
    rpb = np.asarray(inputs['rpb_table'], f32)
    proj_w = np.asarray(inputs['proj_w'], f32); proj_b = np.asarray(inputs['proj_b'], f32)
    g2 = np.asarray(inputs['norm2_w'], f32); b2 = np.asarray(inputs['norm2_b'], f32)
    fc1_w = np.asarray(inputs['fc1_w'], f32); fc1_b = np.asarray(inputs['fc1_b'], f32)
    fc2_w = np.asarray(inputs['fc2_w'], f32); fc2_b = np.asarray(inputs['fc2_b'], f32)

    perm = _build_perm()
    x_win = np.ascontiguousarray(x[perm])
    x_res = x_win + proj_b[None, :]

    Wq = qkv_w[0:C] * g1[None, :] * SCALE
    Wk = qkv_w[C:2 * C] * g1[None, :]
    Wv = qkv_w[2 * C:3 * C] * g1[None, :]
    bq = (qkv_b[0:C] + qkv_w[0:C] @ b1) * SCALE
    bk = qkv_b[C:2 * C] + qkv_w[C:2 * C] @ b1
    bv = qkv_b[2 * C:3 * C] + qkv_w[2 * C:3 * C] @ b1

    w1 = np.zeros((384, C), f32)
    bflat = np.zeros((384,), f32)
    for h in range(NH):
        w1[32 * h:32 * h + HD] = Wq[HD * h:HD * (h + 1)]
        w1[192 + 32 * h:192 + 32 * h + HD] = Wk[HD * h:HD * (h + 1)]
        bflat[32 * h:32 * h + HD] = bq[HD * h:HD * (h + 1)]
        bflat[192 + 32 * h:192 + 32 * h + HD] = bk[HD * h:HD * (h + 1)]
    # 4 M-chunks of 96 rows (3 heads each) so head bases stay in {0,32,64}
    b1c = np.ascontiguousarray(bflat.reshape(4, 96).T)   # (96, 4)
    w1t = np.ascontiguousarray(w1.T)             # (180, 384)

    wv = np.zeros((256, C), f32)
    bvrow = np.zeros((192,), f32)
    for h in range(NH):
        bvrow[32 * h] = 1.0
        wv[32 * h + 1:32 * h + 1 + HD] = Wv[HD * h:HD * (h + 1)]
        bvrow[32 * h + 1:32 * h + 1 + HD] = bv[HD * h:HD * (h + 1)]
    wvt = np.ascontiguousarray(wv.T)             # (180, 256)
    bvrep = np.ascontiguousarray(np.broadcast_to(bvrow[None, :], (128, 192)))

    wp = np.zeros((192, 256), f32)
    for h in range(NH):
        wp[32 * h + 1:32 * h + 1 + HD, 0:C] = proj_w[:, HD * h:HD * (h + 1)].T

    w1m = fc1_w * g2[None, :]
    b1m_flat = fc1_b + fc1_w @ b2
    b1m = np.zeros((128, 6), f32)
    for mc in range(6):
        lo = mc * 128
        hi = min(lo + 128, HID)
        b1m[0:hi - lo, mc] = b1m_flat[lo:hi]
    fc1t = np.ascontiguousarray(w1m.T)           # (180, 720)
    fc2t = np.zeros((721, 256), f32)
    fc2t[0:HID, 0:C] = fc2_w.T
    fc2t[HID, 0:C] = fc2_b

    bias = rpb[rpi]                              # (512,512,NH) [q,k,h]
    bm = bias.transpose(2, 0, 1)[None] + mask[:, None]      # (w,h,q,k)
    bm = bm.reshape(NW, NH, N, 4, 128).transpose(0, 1, 4, 3, 2)
    bm = np.ascontiguousarray(bm.reshape(NW, NH, 128, 4 * N)).astype(ml_dtypes.bfloat16)

    ident = np.eye(128, dtype=f32)
    identb = np.eye(128, dtype=ml_dtypes.bfloat16)
    ones = np.ones((128, 32), f32)
    onesr = np.ones((1, 128), f32)

    return dict(perm=perm, x_win=x_win, x_res=x_res, w1t=w1t, b1c=b1c,
                wvt=wvt, bvrep=bvrep, wp=wp, fc1t=fc1t, b1m=b1m, fc2t=fc2t,
                bm=bm, ident=ident, identb=identb, ones=ones, onesr=onesr)


def _r(ap):
    return ap.bitcast(F32R)


def _build_bass():
    nc = bacc.Bacc("TRN2", target_bir_lowering=False, debug=False,
                   num_devices=NCORES)
    d_xln = nc.dram_tensor("xln", [TPC, C], F32, kind="ExternalInput")
    d_xres = nc.dram_tensor("xres", [TPC, C], F32, kind="ExternalInput")
    d_w1t = nc.dram_tensor("w1t", [C, 384], F32R, kind="ExternalInput")
    d_b1c = nc.dram_tensor("b1c", [96, 4], F32, kind="ExternalInput")
    d_wvt = nc.dram_tensor("wvt", [C, 256], F32R, kind="ExternalInput")
    d_bvrep = nc.dram_tensor("bvrep", [128, 192], F32, kind="ExternalInput")
    d_wp = nc.dram_tensor("wp", [192, 256], F32R, kind="ExternalInput")
    d_fc1t = nc.dram_tensor("fc1t", [C, HID], F32R, kind="ExternalInput")
    d_b1m = nc.dram_tensor("b1m", [128, 6], F32, kind="ExternalInput")
    d_fc2t = nc.dram_tensor("fc2t", [HID + 1, 256], F32R, kind="ExternalInput")
    d_bm = nc.dram_tensor("bm", [WPC, NH, 128, 4 * N], BF16, kind="ExternalInput")
    d_ident = nc.dram_tensor("ident", [128, 128], F32, kind="ExternalInput")
    d_identb = nc.dram_tensor("identb", [128, 128], BF16, kind="ExternalInput")
    d_ones = nc.dram_tensor("ones", [128, 32], F32R, kind="ExternalInput")
    d_onesr = nc.dram_tensor("onesr", [1, 128], F32R, kind="ExternalInput")
    d_y = nc.dram_tensor("y", [TPC, C], F32, kind="ExternalOutput")

    with tile.TileContext(nc) as tc:
        _emit(tc, nc, d_xln, d_xres, d_w1t, d_b1c, d_wvt, d_bvrep, d_wp,
              d_fc1t, d_b1m, d_fc2t, d_bm, d_ident, d_identb, d_ones, d_onesr, d_y)
    nc.compile()
    return nc


def _emit(tc, nc, d_xln, d_xres, d_w1t, d_b1c, d_wvt, d_bvrep, d_wp,
          d_fc1t, d_b1m, d_fc2t, d_bm, d_ident, d_identb, d_ones, d_onesr, d_y):
    from contextlib import ExitStack
    ctx = ExitStack()
    consts = ctx.enter_context(tc.tile_pool(name="consts", bufs=1))
    sb = ctx.enter_context(tc.tile_pool(name="sb", bufs=1))
    psum = ctx.enter_context(tc.tile_pool(name="psum", bufs=1, space="PSUM"))

    def ct(name, shape, dtype, src):
        t = consts.tile(shape, dtype, name=name, tag=name, bufs=1)
        nc.sync.dma_start(t[:], src)
        return t

    w1t_a = ct("w1t_a", [128, 384], F32R, d_w1t[0:128, :])
    w1t_b = ct("w1t_b", [C - 128, 384], F32R, d_w1t[128:C, :])
    wvt_a = ct("wvt_a", [128, 256], F32R, d_wvt[0:128, :])
    wvt_b = ct("wvt_b", [C - 128, 256], F32R, d_wvt[128:C, :])
    b1c = ct("b1c", [96, 4], F32, d_b1c[:, :])
    bvrep = ct("bvrep", [128, 192], F32, d_bvrep[:, :])
    wps = [ct(f"wp{h}", [32, 256], F32R, d_wp[32 * h:32 * (h + 1), :])
           for h in range(NH)]
    fc1t_a = ct("fc1t_a", [128, HID], F32R, d_fc1t[0:128, :])
    fc1t_b = ct("fc1t_b", [C - 128, HID], F32R, d_fc1t[128:C, :])
    b1m = ct("b1m", [128, 6], F32, d_b1m[:, :])
    fc2s = []
    for mc in range(6):
        lo = mc * 128
        hi = min(lo + 128, HID)
        fc2s.append(ct(f"fc2_{mc}", [hi - lo, 256], F32R, d_fc2t[lo:hi, :]))
    fc2b = ct("fc2b", [1, 256], F32R, d_fc2t[HID:HID + 1, :])
    ident = ct("ident", [128, 128], F32, d_ident[:, :])
    identb = ct("identb", [128, 128], BF16, d_identb[:, :])
    ones = ct("ones", [128, 32], F32R, d_ones[:, :])

    # ---------------- persistent SBUF tensors ----------------
    onesr = ct("onesr", [1, 128], F32R, d_onesr[:, :])
    zTa = sb.tile([128, TPC], F32R, name="zTa", tag="zTa", bufs=1)
    zTb = sb.tile([C - 128, TPC], F32R, name="zTb", tag="zTb", bufs=1)

    res_tiles = [sb.tile([128, C], F32, name=f"res{i}", tag=f"res{i}", bufs=1)
                 for i in range(NT)]

    # ---------------- LN phase helper ----------------
    def ln_phase(src_tile_fn, ztag):
        """src_tile_fn(i) -> (128, C) token tile AP (must stay valid for the
        whole phase). Writes standardized activations into zTa/zTb."""
        sums = sb.tile([128, NT], F32, name=f"sums_{ztag}", tag=f"sums_{ztag}", bufs=1)
        sqs = sb.tile([128, NT], F32, name=f"sqs_{ztag}", tag=f"sqs_{ztag}", bufs=1)
        scr2 = sb.tile([128, C], F32, name=f"scr2_{ztag}", tag=f"scr_{ztag}", bufs=1)
        for i in range(NT):
            xt = src_tile_fn(i)
            nc.vector.tensor_reduce(sums[:, i:i + 1], xt[:], axis=AX.X, op=OP.add)
            nc.vector.scalar_tensor_tensor(scr2[:], xt[:], 0.0, xt[:],
                                           OP.bypass, OP.mult,
                                           accum_out=sqs[:, i:i + 1])
        mu = sb.tile([128, NT], F32, name=f"mu_{ztag}", tag=f"mu_{ztag}", bufs=1)
        sfac = sb.tile([128, NT], F32, name=f"s_{ztag}", tag=f"s_{ztag}", bufs=1)
        musq = sb.tile([128, NT], F32, name=f"musq_{ztag}", tag=f"musq_{ztag}", bufs=1)
        vare = sb.tile([128, NT], F32, name=f"vare_{ztag}", tag=f"vare_{ztag}", bufs=1)
        nc.vector.tensor_scalar(mu[:], sums[:], 1.0 / C, None, OP.mult)
        nc.vector.tensor_tensor(musq[:], mu[:], mu[:], OP.mult)
        nc.vector.tensor_scalar(vare[:], sqs[:], 1.0 / C, EPS, OP.mult, OP.add)
        nc.vector.tensor_tensor(vare[:], vare[:], musq[:], OP.subtract)
        inv = sb.tile([128, NT], F32, name=f"inv_{ztag}", tag=f"inv_{ztag}", bufs=1)
        nc.vector.reciprocal(inv[:], vare[:])
        s0 = sb.tile([128, NT], F32, name=f"s0_{ztag}", tag=f"s0_{ztag}", bufs=1)
        nc.scalar.activation(s0[:], inv[:], AF.Sqrt)
        # one Newton step: s = s0 * (1.5 - 0.5 * vare * s0^2)
        t1 = sb.tile([128, NT], F32, name=f"t1_{ztag}", tag=f"t1_{ztag}", bufs=1)
        nc.vector.tensor_tensor(t1[:], s0[:], s0[:], OP.mult)
        nc.vector.tensor_tensor(t1[:], t1[:], vare[:], OP.mult)
        nc.vector.tensor_scalar(t1[:], t1[:], -0.5, 1.5, OP.mult, OP.add)
        nc.vector.tensor_tensor(sfac[:], s0[:], t1[:], OP.mult)

        # z + transpose into zTa/zTb (groups of 4 token tiles)
        for j in range(NT // 4):
            tpA = psum.tile([128, 512], F32, name=f"tpA_{ztag}", tag="big", bufs=2)
            tpB = psum.tile([C - 128, 512], F32, name=f"tpB_{ztag}", tag="pg", bufs=2)
            for t in range(4):
                i = 4 * j + t
                xt = src_tile_fn(i)
                z = sb.tile([128, C], F32, name=f"z_{ztag}", tag="ztok", bufs=6)
                nc.vector.tensor_scalar(z[:], xt[:], mu[:, i:i + 1],
                                        sfac[:, i:i + 1], OP.subtract, OP.mult)
                nc.tensor.transpose(tpA[:, 128 * t:128 * (t + 1)],
                                    z[:, 0:128], ident[:])
                nc.tensor.transpose(tpB[0:C - 128, 128 * t:128 * (t + 1)],
                                    z[:, 128:C], ident[:, 0:128])
            nc.vector.tensor_copy(zTa[:, 512 * j:512 * (j + 1)], tpA[:])
            nc.scalar.activation(zTb[0:C - 128, 512 * j:512 * (j + 1)],
                                 tpB[0:C - 128, :], AF.Copy)

    # ================= P1: LN1 =================
    def xln_tile(i):
        xt = sb.tile([128, C], F32, name="xln", tag="xln", bufs=6)
        nc.sync.dma_start(xt[:], d_xln[128 * i:128 * (i + 1), :])
        return xt

    ln_phase(xln_tile, "ln1")

    STAGE = int(os.environ.get("KBSTAGE", "6"))
    if STAGE < 2:
        ctx.close()
        return

    # ================= P3: attention per window =================
    QCH = ((0, 96), (96, 192), (192, 288), (288, 384))  # GEMM1 M-chunks (3 heads ea)
    for w in range(WPC):
        tok0 = w * N
        # ---- GEMM1: qkT (C-major, 4 chunks) ----
        qk_tiles = []
        for ci, (lo, hi) in enumerate(QCH):
            m = hi - lo
            ps = psum.tile([m, 512], F32, name="g1", tag="pg", bufs=2)
            nc.tensor.matmul(ps[:], _r(w1t_a[:, lo:hi]),
                             _r(zTa[:, tok0:tok0 + N]), start=True, stop=False)
            nc.tensor.matmul(ps[:], _r(w1t_b[:, lo:hi]),
                             _r(zTb[:, tok0:tok0 + N]), start=False, stop=True)
            qt = sb.tile([m, 512], F32R, name=f"qk{ci}", tag=f"qk{ci}", bufs=2)
            nc.scalar.activation(qt[:], ps[:], AF.Identity, bias=b1c[0:m, ci:ci + 1])
            qk_tiles.append(qt)
        # ---- GEMM2: vext (token-major, 4 chunks) ----
        v_tiles = []
        for tch in range(4):
            c0 = tok0 + 128 * tch
            ps = psum.tile([128, 256], F32, name="g2", tag="pg", bufs=2)
            nc.tensor.matmul(ps[:], _r(zTa[:, c0:c0 + 128]), _r(wvt_a[:]),
                             start=True, stop=False)
            nc.tensor.matmul(ps[:], _r(zTb[:, c0:c0 + 128]), _r(wvt_b[:]),
                             start=False, stop=True)
            vt = sb.tile([128, 192], F32R, name="vext", tag="vext", bufs=8)
            nc.vector.tensor_tensor(vt[:], ps[:, 0:192], bvrep[:], OP.add)
            v_tiles.append(vt)
        # ---- heads ----
        if STAGE < 3:
            continue
        nt_tiles = []
        for h in range(NH):
            qt, kt = qk_tiles[h // 3], qk_tiles[2 + h // 3]
            base = 32 * (h % 3)
            qT = qt[base:base + HD, :]
            kT = kt[base:base + HD, :]
            bmt = sb.tile([128, 4 * N], BF16, name="bmt", tag="bmt", bufs=3)
            nc.sync.dma_start(bmt[:], d_bm[w, h, :, :])
            exps = []
            for half in range(2):
                ps = psum.tile([128, 1024], F32, name="pS", tag="big", bufs=2)
                for k2 in range(2):
                    kc = 2 * half + k2
                    sl = slice(512 * k2, 512 * (k2 + 1))
                    nc.tensor.matmul(ps[:, sl], _r(kT[:, 128 * kc:128 * (kc + 1)]),
                                     _r(qT[:]), start=True, stop=False)
                    nc.tensor.matmul(ps[:, sl], identb[:],
                                     bmt[:, 512 * kc:512 * (kc + 1)],
                                     start=False, stop=True)
                ex = sb.tile([128, 1024], F32R, name="exps", tag="exps", bufs=3)
                nc.scalar.activation(ex[:], ps[:], AF.Exp)
                exps.append(ex)
            po = psum.tile([32, 512], F32, name="po", tag="po", bufs=2)
            for kc in range(4):
                nc.tensor.matmul(po[:],
                                 _r(v_tiles[kc][:, 32 * h:32 * (h + 1)]),
                                 _r(exps[kc // 2][:, 512 * (kc % 2):512 * (kc % 2 + 1)]),
                                 start=(kc == 0), stop=(kc == 3))
            if STAGE < 4:
                continue
            # normalize: row 0 of po is the softmax denominator
            rcrow = sb.tile([1, 512], F32R, name="rcrow", tag="rcrow", bufs=4)
            with nc.allow_low_precision(reason="softmax denom recip (f32r)"):
                nc.vector.reciprocal(rcrow[:], po[0:1, :])
            rb = psum.tile([32, 512], F32, name="rb", tag="po", bufs=2)
            nc.tensor.matmul(rb[:], _r(ones[0:1, :]), rcrow[:],
                             start=True, stop=True)
            rbs = sb.tile([32, 512], F32, name="rbs", tag="rbs", bufs=2)
            nc.scalar.activation(rbs[:], rb[:], AF.Copy)
            ntt = sb.tile([32, 512], F32R, name="ntt", tag="ntt", bufs=8)
            nc.vector.tensor_tensor(ntt[:], po[:], rbs[:], OP.mult)
            nt_tiles.append(ntt)
        # ---- proj + residual ----
        if STAGE < 4:
            continue
        for tch in range(4):
            i = 4 * w + tch
            xr = sb.tile([128, C], F32, name="xres", tag="xres", bufs=6)
            nc.sync.dma_start(xr[:], d_xres[128 * i:128 * (i + 1), :])
            ps = psum.tile([128, 256], F32, name="pp", tag="pg", bufs=2)
            for h in range(NH):
                nc.tensor.matmul(ps[:],
                                 _r(nt_tiles[h][:, 128 * tch:128 * (tch + 1)]),
                                 _r(wps[h][:]),
                                 start=(h == 0), stop=(h == NH - 1))
            nc.vector.tensor_tensor(res_tiles[i][:], ps[:, 0:C], xr[:], OP.add)

    if STAGE < 5:
        ctx.close()
        return

    # ================= P4: LN2 =================
    ln_phase(lambda i: res_tiles[i], "ln2")

    # ================= P5: MLP per window =================
    if STAGE < 6:
        ctx.close()
        return
    MCH = tuple((mc * 128, min(mc * 128 + 128, HID)) for mc in range(6))
    for w in range(WPC):
        tok0 = w * N
        h_tiles = []
        for mc, (lo, hi) in enumerate(MCH):
            m = hi - lo
            ps = psum.tile([m, 512], F32, name="f1", tag="big", bufs=2)
            nc.tensor.matmul(ps[:], _r(fc1t_a[:, lo:hi]),
                             _r(zTa[:, tok0:tok0 + N]), start=True, stop=False)
            nc.tensor.matmul(ps[:], _r(fc1t_b[:, lo:hi]),
                             _r(zTb[:, tok0:tok0 + N]), start=False, stop=True)
            ht = sb.tile([m, 512], F32R, name=f"h{mc}", tag=f"h{mc}", bufs=2)
            nc.scalar.activation(ht[:], ps[:], AF.Gelu, bias=b1m[0:m, mc:mc + 1])
            h_tiles.append(ht)
        for tch in range(4):
            i = 4 * w + tch
            sl = slice(128 * tch, 128 * (tch + 1))
            ps = psum.tile([128, 256], F32, name="f2", tag="pg", bufs=2)
            for mc in range(6):
                hk = h_tiles[mc].shape[0]
                nc.tensor.matmul(ps[:], _r(h_tiles[mc][0:hk, sl]),
                                 _r(fc2s[mc][0:hk, :]),
                                 start=(mc == 0), stop=False)
            nc.tensor.matmul(ps[:], _r(onesr[0:1, 0:128]), _r(fc2b[:]),
                             start=False, stop=True)
            yt = sb.tile([128, C], F32, name="yt", tag="ztok", bufs=6)
            nc.vector.tensor_tensor(yt[:], ps[:, 0:C], res_tiles[i][:], OP.add)
            nc.sync.dma_start(d_y[128 * i:128 * (i + 1), :], yt[:])

    ctx.close()


_CACHED = {}


def _get_bass():
    if "nc" not in _CACHED:
        _CACHED["nc"] = _build_bass()
    return _CACHED["nc"]


def kernel(**inputs) -> np.ndarray:
    p = _host_prep(inputs)
    nc = _get_bass()
    shared = dict(w1t=p['w1t'], b1c=p['b1c'], wvt=p['wvt'], bvrep=p['bvrep'],
                  wp=p['wp'], fc1t=p['fc1t'], b1m=p['b1m'], fc2t=p['fc2t'],
                  ident=p['ident'], identb=p['identb'], ones=p['ones'], onesr=p['onesr'])
    in_maps = []
    for c in range(NCORES):
        m = dict(shared)
        m['xln'] = p['x_win'][c * TPC:(c + 1) * TPC]
        m['xres'] = p['x_res'][c * TPC:(c + 1) * TPC]
        m['bm'] = p['bm'][c * WPC:(c + 1) * WPC]
        in_maps.append(m)
    r = bass_utils.run_bass_kernel_spmd(nc, in_maps, core_ids=list(range(NCORES)))
    _CACHED["last_results"] = r
    y_win = np.concatenate([r.results[c]['y'] for c in range(NCORES)], axis=0)
    y_full = np.empty((H * W * D, C), np.float32)
    y_full[p['perm']] = y_win
    return y_full.reshape(1, H * W * D, C)
